# revision 16
# baseline (speedup 1.0000x reference)
"""Dense all-expert MoE (SwiGLU) kernel for Trainium2, expert-parallel over 8 cores.

Computes: out = sum_e silu(x @ Wg[e]) * (x @ Wu[e]) @ Wd[e]
with x: [B=2, S=2048, H=1024], Wg/Wu: [8, 1024, 4096], Wd: [8, 4096, 1024].

Sharding: expert-parallel. Core e gets expert e's weights plus the full token
set; each core produces a partial [T, H] output which the host sums.

The kernel sits at the PE roofline (fp16 matmul = 1 cycle/row; measured MM
issue interval 216.8ns for [128,512]). fp8e4 DoubleRow (K=256/instr) is 2x
the MAC rate but costs ~0.44%% of end-to-end rel-err per converted 128-wide
ffn slice (e4m3 quantization of both operands), so the 2e-2 error gate
bounds how much of the kernel can ride it:

  - base precision is fp16 (same speed as bf16, 4x less rounding error:
    0.16%% vs 0.45%% end-to-end) to maximize the budget left for fp8;
  - the last 8 ffn slices (f=24..31) of the down-projection run as fp8
    DoubleRow pairs: 4 DR matmuls replace 8 fp16 ones per accumulator
    (-54.6us of PE time vs all-fp16). Sim-exact predicted rel err 1.877%%.

All stage-B chains accumulate in ONE psum bank at a UNIFORM product scale
s_h*s_d = 16384: h tiles are written as h*8 (fp16 or e4m3) by the DVE
scalar_tensor_tensor (sil*8)*u, and Wd is host-scaled x2048 in both fp16
and e4m3 forms. Eviction is a single Act copy with scale 1/16384 - no
spare-bank rescale/add machinery.

Other perf notes (carried from NTFF traces of earlier versions):
  - PE warmup: 28 dummy [128,128] matmuls run while the cold-start DMAs
    land (PE would idle 7..10us otherwise), so the DVFS p-state is at
    full clock when the first real matmul issues (~3us saved). The count
    matters: warmups ahead of the first real matmul in PE program order
    delay it once its operands have landed (~10us in).
  - Cold start splits the first weight/x transfers finely (32KB + 128KB
    gate the first matmul instead of 512KB) and interleaves both rings in
    PE consumption order; later chunks escalate in size as rings ramp.
  - All DMA rides the sync/scalar HWDGE rings (gpsimd SWDGE costs a ~4.3us
    drain at kernel end).
  - Stage B runs m-outer/f-inner so each accumulator finishes and evicts
    while the next accumulates; the tail after the last matmul is one
    [128,512] copy + one 128KB DMA.
  - x prefetch for block t+1 is issued on sync before stage B's out DMAs;
    its WAR on the single-buffered x tile resolves at stage-B start so the
    2MB transfer overlaps stage B.
  - h8 pair tiles are written [:, j, sl] by the DVE; the DR moving operands
    (wd8 tiles) are full [128,2,512] tiles (sliced 3D moving APs fall off
    the PE fast path).
"""

import numpy as np
import ml_dtypes

T = 4096          # B*S tokens
H = 1024          # hidden
F = 4096          # ffn
E = 8
N_CORES = 8
TB = 1024         # tokens per block
NT = T // TB      # 4 token blocks
KB = H // 128     # 8 hidden slices
FB = F // 128     # 32 ffn slices
FQ = 4            # f-slices per stage-A weight tile
NF8 = 8           # trailing ffn slices in fp8 (must be even)
NP8 = NF8 // 2    # fp8 pairs
FB16 = FB - NF8   # leading ffn slices in fp16 (24)

S_H = 8.0         # h scale folded into the DVE stt (|h|max ~22 -> 175)
S_D = 2048.0      # Wd scale (|Wd|max ~0.086 -> 176)
EVICT = 1.0 / (S_H * S_D)

_CACHE = {}


def _build_module():
    from contextlib import ExitStack

    import concourse.bass as bass
    import concourse.mybir as mybir
    import concourse.tile as tile
    from concourse import bacc

    f32 = mybir.dt.float32
    f16 = mybir.dt.float16
    fp8 = mybir.dt.float8e4
    DR = mybir.MatmulPerfMode.DoubleRow
    MULT = mybir.AluOpType.mult

    nc = bacc.Bacc(
        "TRN2",
        target_bir_lowering=False,
        debug=False,
        enable_asserts=False,
        num_devices=N_CORES,
    )

    # xQ[t, p, k, tt] = x[t*TB+tt, 128k+p]
    xQ = nc.dram_tensor("xQ", [NT, 128, KB, TB], f16, kind="ExternalInput").ap()
    # wQ[i, p, 0, f2, k*128+m] = Wg[128k+p, 128(4i+f2)+m]; [.., 1, ..] = Wu
    wQ = nc.dram_tensor(
        "wQ", [FB // FQ, 128, 2, FQ, KB * 128], f16, kind="ExternalInput"
    ).ap()
    # wdQ[h2, p, f*512+h] = Wd[128f+p, 512h2+h] * S_D, f < 24
    wdQ = nc.dram_tensor("wdQ", [2, 128, FB16 * 512], f16,
                         kind="ExternalInput").ap()
    # wd8Q[pr, h2, p, j, hh] = e4m3(Wd[128*(24+2pr+j)+p, 512h2+hh] * S_D)
    wd8Q = nc.dram_tensor("wd8Q", [NP8, 2, 128, 2, 512], fp8,
                          kind="ExternalInput").ap()
    out = nc.dram_tensor("out", [T, H], f16, kind="ExternalOutput").ap()

    # wd preload schedule: stage-A weight-tile index -> wdQ (h2, col half)
    wd_sched = {8: (0, 0), 12: (0, 1), 16: (1, 0), 20: (1, 1)}

    with tile.TileContext(nc) as tc, ExitStack() as ctx:
        xpool = ctx.enter_context(tc.tile_pool(name="xpool", bufs=1))
        wpool = ctx.enter_context(tc.tile_pool(name="wpool", bufs=2))
        dpool = ctx.enter_context(tc.tile_pool(name="dpool", bufs=1))
        hpool = ctx.enter_context(tc.tile_pool(name="hpool", bufs=1))
        spool = ctx.enter_context(tc.tile_pool(name="spool", bufs=2))
        opool = ctx.enter_context(tc.tile_pool(name="opool", bufs=3))
        cpool = ctx.enter_context(tc.tile_pool(name="cpool", bufs=1))
        # one psum pool, 8 single-bank [128,512] tags: per-bank tiles keep
        # Tile's WAR tracking at bank granularity.
        # stage A even f: g->(b0,b1) u->(b2,b3); odd f: b4..b7 (c halves).
        # stage B: accumulator m -> b{m} (single bank, fp16+fp8 chains).
        psum = ctx.enter_context(tc.tile_pool(name="psum", bufs=1, space="PSUM"))

        bias0 = cpool.tile([128, 1], f32, tag="bias0")
        nc.vector.memset(bias0[:], 0.0)

        # PE p-state warmup: dummy matmuls keep the PE continuously busy
        # while the cold-start DMAs land, so the first real matmul runs at
        # the full 2.4GHz p-state instead of paying the ramp.
        warm_w = cpool.tile([128, 128], f16, tag="warm_w")
        nc.vector.memset(warm_w[:], 0.0)
        warm_p = psum.tile([128, 128], f32, tag="b7", name="warm")
        # 28 x ~107ns ends right as the first real matmul's operands land;
        # more would delay it (PE executes in program order)
        for _ in range(28):
            nc.tensor.matmul(warm_p[:], warm_w[:], warm_w[:],
                             start=True, stop=True)

        # Wd stays resident in SBUF for the whole kernel
        wdp = [
            dpool.tile([128, FB16 * 512], f16, tag=f"wdp{h2}", name=f"wdp{h2}")
            for h2 in range(2)
        ]
        wd8p = [
            [
                dpool.tile([128, 2, 512], fp8, tag=f"wd8_{pr}_{h2}",
                           name=f"wd8_{pr}_{h2}")
                for h2 in range(2)
            ]
            for pr in range(NP8)
        ]

        xbs = {}
        for t in range(NT):
            # ---- stage A: hT[f] = silu(Wg_f^T xT) * (Wu_f^T xT), F on partitions
            if t == 0:
                # cold start: both HWDGE rings ramp slowly, so the first
                # block's x and weights are issued as a deadline-ordered
                # interleave across BOTH rings, with the two chunks gating
                # the FIRST matmul (g f0 k0 c0) split off small.
                xb = xpool.tile([128, KB, TB], f16, tag="xb")
                wt0 = wpool.tile([128, 2, FQ, KB * 128], f16, tag="w")
                # the two chunks gating the first matmul (g f0 k0 c0) ship
                # first and small; afterwards both rings carry a balanced
                # byte load (sync: g chunks, scalar: x + u chunks) in PE
                # consumption order with escalating sizes
                nc.sync.dma_start(wt0[:, 0, 0, 0:128], wQ[0][:, 0, 0, 0:128])
                nc.scalar.dma_start(xb[:, 0, 0:512], xQ[0][:, 0, 0:512])
                nc.sync.dma_start(wt0[:, 0, 0, 128:384], wQ[0][:, 0, 0, 128:384])
                nc.scalar.dma_start(xb[:, 0, 512:], xQ[0][:, 0, 512:])
                nc.sync.dma_start(wt0[:, 0, 0, 384:], wQ[0][:, 0, 0, 384:])
                nc.scalar.dma_start(xb[:, 1, :], xQ[0][:, 1, :])
                # per-k x DMAs: the g-chain's matmul pair for slice k gates
                # on its OWN 256KB transfer instead of a merged 1MB one
                # (packets stay 2KB/partition either way; only the completion
                # semaphore granularity changes)
                nc.scalar.dma_start(xb[:, 2, :], xQ[0][:, 2, :])
                nc.scalar.dma_start(xb[:, 3, :], xQ[0][:, 3, :])
                nc.sync.dma_start(wt0[:, 1, 0, :], wQ[0][:, 1, 0, :])  # u f0
                for k in range(4, KB):
                    nc.scalar.dma_start(xb[:, k, :], xQ[0][:, k, :])
                nc.sync.dma_start(wt0[:, 0, 1, :], wQ[0][:, 0, 1, :])  # g f1
                nc.scalar.dma_start(wt0[:, 1, 1, :], wQ[0][:, 1, 1, :])  # u f1
                nc.sync.dma_start(wt0[:, 0, 2:FQ, :], wQ[0][:, 0, 2:FQ, :])
                nc.scalar.dma_start(wt0[:, 1, 2:FQ, :], wQ[0][:, 1, 2:FQ, :])
                wts = {0: wt0}
            else:
                xb = xbs.pop(t)

            hts = []
            ht8s = {}
            for fq in range(0, FB, FQ):
                # combined wg+wu tile for FQ f-slices: ONE PE sem-wait per
                # FQ slices, 16KB-contiguous per partition
                qi = fq // FQ
                if t == 0 and fq == 0:
                    wt = wts.pop(0)  # cold-start ladder issued above
                else:
                    wt = wpool.tile([128, 2, FQ, KB * 128], f16, tag="w")
                    nc.sync.dma_start(wt[:], wQ[qi])
                if t == 0 and fq in wd_sched:
                    # wd preload spread over mid-block tiles: off the critical
                    # cold-start path, done before stage B needs each half
                    h2i, ci = wd_sched[fq]
                    sl = slice(ci * FB16 * 256, (ci + 1) * FB16 * 256)
                    nc.scalar.dma_start(wdp[h2i][:, sl], wdQ[h2i][:, sl])
                if t == 0 and fq == 24:
                    for pr in range(NP8):
                        for h2i in range(2):
                            nc.scalar.dma_start(
                                wd8p[pr][h2i][:], wd8Q[pr, h2i]
                            )

                for f2 in range(FQ):
                    f = fq + f2
                    b = (f % 2) * 4  # psum banks: even f -> b0..b3, odd -> b4..b7
                    sil = spool.tile([128, TB], f32, tag="sil")
                    # g runs k-outer/c-inner: each xb k-chunk is consumed for
                    # two matmuls before the next is needed (cold start feeds
                    # at half the rate of a c-outer loop)
                    gc = [
                        psum.tile(
                            [128, 512], f32, tag=f"b{b + c}", name=f"g{t}_{f}_{c}"
                        )
                        for c in range(TB // 512)
                    ]
                    for k in range(KB):
                        for c in range(TB // 512):
                            nc.tensor.matmul(
                                gc[c][:],
                                wt[:, 0, f2, k * 128 : (k + 1) * 128],
                                xb[:, k, c * 512 : (c + 1) * 512],
                                start=(k == 0),
                                stop=(k == KB - 1),
                            )
                    for c in range(TB // 512):
                        sl = slice(c * 512, (c + 1) * 512)
                        nc.scalar.activation(
                            sil[:, sl], gc[c][:], mybir.ActivationFunctionType.Silu,
                            bias=bias0[:],
                        )

                    # u runs c-outer so each 512-col half of the product is
                    # ready as soon as its 8 k-accumulation matmuls retire
                    if f >= FB16:
                        pr, j = (f - FB16) // 2, (f - FB16) % 2
                        if j == 0:
                            ht8s[pr] = hpool.tile(
                                [128, 2, TB], fp8, tag=f"h8_{pr}",
                                name=f"h8_{t}_{pr}",
                            )
                        ht = None
                    else:
                        ht = hpool.tile([128, TB], f16, tag=f"h{f}")
                    for c in range(TB // 512):
                        sl = slice(c * 512, (c + 1) * 512)
                        u = psum.tile([128, 512], f32, tag=f"b{b + 2 + c}")
                        for k in range(KB):
                            nc.tensor.matmul(
                                u[:],
                                wt[:, 1, f2, k * 128 : (k + 1) * 128],
                                xb[:, k, sl],
                                start=(k == 0),
                                stop=(k == KB - 1),
                            )
                        # h tile = (sil * S_H) * u, written at the uniform
                        # stage-B operand scale in one DVE op
                        dst = ht[:, sl] if ht is not None else ht8s[pr][:, j, sl]
                        nc.vector.scalar_tensor_tensor(
                            dst, sil[:, sl], S_H, u[:], MULT, MULT,
                        )
                    if ht is not None:
                        hts.append(ht)

            if t + 1 < NT:
                # prefetch next block's x now: the trigger lands on the sync
                # ring AHEAD of this block's output-DMA triggers; its WAR on
                # the single-buffered tile resolves as stage B starts
                nxb = xpool.tile([128, KB, TB], f16, tag="xb")
                nc.sync.dma_start(nxb[:], xQ[t + 1])
                xbs[t + 1] = nxb

            # ---- stage B: out[tokens, h] += hT^T @ Wd, tokens on partitions
            # m-outer/f-inner; ALL chains (24 fp16 + 4 fp8 DoubleRow) land in
            # one bank at scale S_H*S_D, evicted by a single scaled Act copy
            for h2 in range(2):
                for m in range(8):
                    acc = psum.tile(
                        [128, 512], f32, tag=f"b{m}", name=f"acc{t}_{h2}_{m}"
                    )
                    msl = slice(m * 128, (m + 1) * 128)
                    for f in range(FB16):
                        nc.tensor.matmul(
                            acc[:],
                            hts[f][:, msl],
                            wdp[h2][:, f * 512 : (f + 1) * 512],
                            start=(f == 0),
                            stop=False,
                        )
                    for pr in range(NP8):
                        nc.tensor.matmul(
                            acc[:],
                            ht8s[pr][:, :, msl],
                            wd8p[pr][h2][:],
                            start=False,
                            stop=(pr == NP8 - 1),
                            perf_mode=DR,
                        )
                    ob = opool.tile([128, 512], f16, tag="ob")
                    nc.scalar.activation(
                        ob[:], acc[:], mybir.ActivationFunctionType.Copy,
                        scale=EVICT,
                    )
                    row = t * TB + m * 128
                    dst = out[row : row + 128, h2 * 512 : (h2 + 1) * 512]
                    eng = nc.scalar if m < 4 else nc.sync
                    eng.dma_start(dst, ob[:])

    nc.compile()
    return nc


def _get_module():
    if "nc" not in _CACHE:
        _CACHE["nc"] = _build_module()
    return _CACHE["nc"]


def _prep_inputs(hidden_states, Wg, Wu, Wd):
    f16 = np.float16
    x = np.asarray(hidden_states, dtype=np.float32).reshape(T, H)
    # xQ[t, p, k, tt] = x[t*TB+tt, 128k+p]
    xQ = np.ascontiguousarray(
        x.reshape(NT, TB, KB, 128).transpose(0, 3, 2, 1)
    ).astype(f16)
    in_maps = []
    for e in range(N_CORES):
        # w[f, p, (k m)] = W[e, 128k+p, 128f+m], f-major tiles of FQ slices
        def _wslices(W):
            return (
                np.asarray(W, dtype=np.float32)
                .reshape(KB, 128, FB, 128)
                .transpose(2, 1, 0, 3)
                .reshape(FB // FQ, FQ, 128, KB * 128)
                .transpose(0, 2, 1, 3)  # [8, 128, FQ, 1024]
            )
        wQ = np.ascontiguousarray(
            np.stack([_wslices(Wg[e]), _wslices(Wu[e])], axis=2)
        ).astype(f16)  # [8, 128, 2, FQ, 1024]
        wd_s = np.asarray(Wd[e], dtype=np.float32) * S_D
        wd_r = wd_s.reshape(FB, 128, 2, 512)
        # wdQ[h2, p, f*512+h] = Wd[e, 128f+p, 512h2+h] * S_D  (f < FB16)
        wdQ = np.ascontiguousarray(
            wd_r[:FB16].transpose(2, 1, 0, 3).reshape(2, 128, FB16 * 512)
        ).astype(f16)
        # wd8Q[pr, h2, p, j, hh] = e4m3 of the last NF8 slices
        wd8Q = np.clip(
            np.ascontiguousarray(
                wd_r[FB16:].reshape(NP8, 2, 128, 2, 512).transpose(0, 3, 2, 1, 4)
            ),
            -240.0, 240.0,
        ).astype(ml_dtypes.float8_e4m3)
        in_maps.append({"xQ": xQ, "wQ": wQ, "wdQ": wdQ, "wd8Q": wd8Q})
    return in_maps


def _run(in_maps, trace=False, **kwargs):
    from concourse import bass_utils

    nc = _get_module()
    return bass_utils.run_bass_kernel_spmd(
        nc, in_maps, core_ids=list(range(N_CORES)), trace=trace, **kwargs
    )


def kernel(hidden_states, Wg, Wu, Wd):
    import time

    in_maps = _prep_inputs(hidden_states, Wg, Wu, Wd)
    last_exc = None
    for attempt in range(3):
        try:
            res = _run(in_maps)
            break
        except Exception as exc:  # transient device-unrecoverable wedges
            last_exc = exc
            time.sleep(5 * (attempt + 1))
    else:
        raise last_exc
    partials = np.stack(
        [np.asarray(r["out"], dtype=np.float32) for r in res.results], axis=0
    )
    total = partials.sum(axis=0, dtype=np.float32)
    return total.reshape(2, 2048, H).astype(np.float32)


# revision 17
# speedup vs baseline: 1.2000x; 1.2000x over previous
"""Dense all-expert MoE (SwiGLU) kernel for Trainium2, expert-parallel over 8 cores.

Computes: out = sum_e silu(x @ Wg[e]) * (x @ Wu[e]) @ Wd[e]
with x: [B=2, S=2048, H=1024], Wg/Wu: [8, 1024, 4096], Wd: [8, 4096, 1024].

Sharding: expert-parallel. Core e gets expert e's weights plus the full token
set; each core produces a partial [T, H] output which the host sums.

The kernel sits at the PE roofline (fp16 matmul = 1 cycle/row; measured MM
issue interval 216.8ns for [128,512]). fp8e4 DoubleRow (K=256/instr) is 2x
the MAC rate but costs ~0.44%% of end-to-end rel-err per converted 128-wide
ffn slice (e4m3 quantization of both operands), so the 2e-2 error gate
bounds how much of the kernel can ride it:

  - base precision is fp16 (same speed as bf16, 4x less rounding error:
    0.16%% vs 0.45%% end-to-end) to maximize the budget left for fp8;
  - the last 8 ffn slices (f=24..31) of the down-projection run as fp8
    DoubleRow pairs: 4 DR matmuls replace 8 fp16 ones per accumulator
    (-54.6us of PE time vs all-fp16). Sim-exact predicted rel err 1.877%%.

All stage-B chains accumulate in ONE psum bank at a UNIFORM product scale
s_h*s_d = 16384: h tiles are written as h*8 (fp16 or e4m3) by the DVE
scalar_tensor_tensor (sil*8)*u, and Wd is host-scaled x2048 in both fp16
and e4m3 forms. Eviction is a single Act copy with scale 1/16384 - no
spare-bank rescale/add machinery.

Other perf notes (carried from NTFF traces of earlier versions):
  - PE warmup: 28 dummy [128,128] matmuls run while the cold-start DMAs
    land (PE would idle 7..10us otherwise), so the DVFS p-state is at
    full clock when the first real matmul issues (~3us saved). The count
    matters: warmups ahead of the first real matmul in PE program order
    delay it once its operands have landed (~10us in).
  - Cold start splits the first weight/x transfers finely (32KB + 128KB
    gate the first matmul instead of 512KB) and interleaves both rings in
    PE consumption order; later chunks escalate in size as rings ramp.
  - All DMA rides the sync/scalar HWDGE rings (gpsimd SWDGE costs a ~4.3us
    drain at kernel end).
  - Stage B runs m-outer/f-inner so each accumulator finishes and evicts
    while the next accumulates; the tail after the last matmul is one
    [128,512] copy + one 128KB DMA.
  - x prefetch for block t+1 is issued on sync before stage B's out DMAs;
    its WAR on the single-buffered x tile resolves at stage-B start so the
    2MB transfer overlaps stage B.
  - h8 pair tiles are written [:, j, sl] by the DVE; the DR moving operands
    (wd8 tiles) are full [128,2,512] tiles (sliced 3D moving APs fall off
    the PE fast path).
"""

import numpy as np
import ml_dtypes

T = 4096          # B*S tokens
H = 1024          # hidden
F = 4096          # ffn
E = 8
N_CORES = 8
TB = 1024         # tokens per block
NT = T // TB      # 4 token blocks
KB = H // 128     # 8 hidden slices
FB = F // 128     # 32 ffn slices
FQ = 4            # f-slices per stage-A weight tile
NF8 = 8           # trailing ffn slices in fp8 (must be even)
NP8 = NF8 // 2    # fp8 pairs
FB16 = FB - NF8   # leading ffn slices in fp16 (24)

S_H = 8.0         # h scale folded into the DVE stt (|h|max ~22 -> 175)
S_D = 2048.0      # Wd scale (|Wd|max ~0.086 -> 176)
EVICT = 1.0 / (S_H * S_D)

_CACHE = {}


def _build_module():
    from contextlib import ExitStack

    import concourse.bass as bass
    import concourse.mybir as mybir
    import concourse.tile as tile
    from concourse import bacc

    f32 = mybir.dt.float32
    f16 = mybir.dt.float16
    fp8 = mybir.dt.float8e4
    DR = mybir.MatmulPerfMode.DoubleRow
    MULT = mybir.AluOpType.mult

    nc = bacc.Bacc(
        "TRN2",
        target_bir_lowering=False,
        debug=False,
        enable_asserts=False,
        num_devices=N_CORES,
    )

    # xQ[t, p, k, tt] = x[t*TB+tt, 128k+p]
    xQ = nc.dram_tensor("xQ", [NT, 128, KB, TB], f16, kind="ExternalInput").ap()
    # wQ[i, p, 0, f2, k*128+m] = Wg[128k+p, 128(4i+f2)+m]; [.., 1, ..] = Wu
    wQ = nc.dram_tensor(
        "wQ", [FB // FQ, 128, 2, FQ, KB * 128], f16, kind="ExternalInput"
    ).ap()
    # wdQ[h2, p, f*512+h] = Wd[128f+p, 512h2+h] * S_D, f < 24
    wdQ = nc.dram_tensor("wdQ", [2, 128, FB16 * 512], f16,
                         kind="ExternalInput").ap()
    # wd8Q[pr, h2, p, j, hh] = e4m3(Wd[128*(24+2pr+j)+p, 512h2+hh] * S_D)
    wd8Q = nc.dram_tensor("wd8Q", [NP8, 2, 128, 2, 512], fp8,
                          kind="ExternalInput").ap()
    out = nc.dram_tensor("out", [T, H], f16, kind="ExternalOutput").ap()

    # wd preload schedule: stage-A weight-tile index -> wdQ (h2, col half)
    wd_sched = {8: (0, 0), 12: (0, 1), 16: (1, 0), 20: (1, 1)}

    with tile.TileContext(nc) as tc, ExitStack() as ctx:
        xpool = ctx.enter_context(tc.tile_pool(name="xpool", bufs=1))
        wpool = ctx.enter_context(tc.tile_pool(name="wpool", bufs=2))
        dpool = ctx.enter_context(tc.tile_pool(name="dpool", bufs=1))
        hpool = ctx.enter_context(tc.tile_pool(name="hpool", bufs=1))
        spool = ctx.enter_context(tc.tile_pool(name="spool", bufs=2))
        opool = ctx.enter_context(tc.tile_pool(name="opool", bufs=3))
        cpool = ctx.enter_context(tc.tile_pool(name="cpool", bufs=1))
        # one psum pool, 8 single-bank [128,512] tags: per-bank tiles keep
        # Tile's WAR tracking at bank granularity.
        # stage A even f: g->(b0,b1) u->(b2,b3); odd f: b4..b7 (c halves).
        # stage B: accumulator m -> b{m} (single bank, fp16+fp8 chains).
        psum = ctx.enter_context(tc.tile_pool(name="psum", bufs=1, space="PSUM"))

        bias0 = cpool.tile([128, 1], f32, tag="bias0")
        nc.vector.memset(bias0[:], 0.0)

        # PE p-state warmup: dummy matmuls keep the PE continuously busy
        # while the cold-start DMAs land, so the first real matmul runs at
        # the full 2.4GHz p-state instead of paying the ramp.
        warm_w = cpool.tile([128, 128], f16, tag="warm_w")
        nc.vector.memset(warm_w[:], 0.0)
        warm_p = psum.tile([128, 128], f32, tag="b7", name="warm")
        # 28 x ~107ns ends right as the first real matmul's operands land;
        # more would delay it (PE executes in program order)
        for _ in range(28):
            nc.tensor.matmul(warm_p[:], warm_w[:], warm_w[:],
                             start=True, stop=True)

        # Wd stays resident in SBUF for the whole kernel
        wdp = [
            dpool.tile([128, FB16 * 512], f16, tag=f"wdp{h2}", name=f"wdp{h2}")
            for h2 in range(2)
        ]
        wd8p = [
            [
                dpool.tile([128, 2, 512], fp8, tag=f"wd8_{pr}_{h2}",
                           name=f"wd8_{pr}_{h2}")
                for h2 in range(2)
            ]
            for pr in range(NP8)
        ]

        xbs = {}
        for t in range(NT):
            # ---- stage A: hT[f] = silu(Wg_f^T xT) * (Wu_f^T xT), F on partitions
            if t == 0:
                # cold start: both HWDGE rings ramp slowly, so the first
                # block's x and weights are issued as a deadline-ordered
                # interleave across BOTH rings, with the two chunks gating
                # the FIRST matmul (g f0 k0 c0) split off small.
                xb = xpool.tile([128, KB, TB], f16, tag="xb")
                wt0 = wpool.tile([128, 2, FQ, KB * 128], f16, tag="w")
                # the two chunks gating the first matmul (g f0 k0 c0) ship
                # first and small; afterwards both rings carry a balanced
                # byte load (sync: g chunks, scalar: x + u chunks) in PE
                # consumption order with escalating sizes
                nc.sync.dma_start(wt0[:, 0, 0, 0:128], wQ[0][:, 0, 0, 0:128])
                nc.scalar.dma_start(xb[:, 0, 0:512], xQ[0][:, 0, 0:512])
                nc.sync.dma_start(wt0[:, 0, 0, 128:384], wQ[0][:, 0, 0, 128:384])
                nc.scalar.dma_start(xb[:, 0, 512:], xQ[0][:, 0, 512:])
                nc.sync.dma_start(wt0[:, 0, 0, 384:], wQ[0][:, 0, 0, 384:])
                nc.scalar.dma_start(xb[:, 1, :], xQ[0][:, 1, :])
                nc.scalar.dma_start(xb[:, 2:4, :], xQ[0][:, 2:4, :])
                nc.sync.dma_start(wt0[:, 1, 0, :], wQ[0][:, 1, 0, :])  # u f0
                nc.scalar.dma_start(xb[:, 4:KB, :], xQ[0][:, 4:KB, :])
                nc.sync.dma_start(wt0[:, 0, 1, :], wQ[0][:, 0, 1, :])  # g f1
                nc.scalar.dma_start(wt0[:, 1, 1, :], wQ[0][:, 1, 1, :])  # u f1
                nc.sync.dma_start(wt0[:, 0, 2:FQ, :], wQ[0][:, 0, 2:FQ, :])
                nc.scalar.dma_start(wt0[:, 1, 2:FQ, :], wQ[0][:, 1, 2:FQ, :])
                wts = {0: wt0}
            else:
                xb = xbs.pop(t)

            hts = []
            ht8s = {}
            for fq in range(0, FB, FQ):
                # combined wg+wu tile for FQ f-slices: ONE PE sem-wait per
                # FQ slices, 16KB-contiguous per partition
                qi = fq // FQ
                if t == 0 and fq == 0:
                    wt = wts.pop(0)  # cold-start ladder issued above
                else:
                    wt = wpool.tile([128, 2, FQ, KB * 128], f16, tag="w")
                    nc.sync.dma_start(wt[:], wQ[qi])
                if t == 0 and fq in wd_sched:
                    # wd preload spread over mid-block tiles: off the critical
                    # cold-start path, done before stage B needs each half
                    h2i, ci = wd_sched[fq]
                    sl = slice(ci * FB16 * 256, (ci + 1) * FB16 * 256)
                    nc.scalar.dma_start(wdp[h2i][:, sl], wdQ[h2i][:, sl])
                if t == 0 and fq == 24:
                    for pr in range(NP8):
                        for h2i in range(2):
                            nc.scalar.dma_start(
                                wd8p[pr][h2i][:], wd8Q[pr, h2i]
                            )

                for f2 in range(FQ):
                    f = fq + f2
                    b = (f % 2) * 4  # psum banks: even f -> b0..b3, odd -> b4..b7
                    sil = spool.tile([128, TB], f32, tag="sil")
                    # g runs k-outer/c-inner: each xb k-chunk is consumed for
                    # two matmuls before the next is needed (cold start feeds
                    # at half the rate of a c-outer loop)
                    gc = [
                        psum.tile(
                            [128, 512], f32, tag=f"b{b + c}", name=f"g{t}_{f}_{c}"
                        )
                        for c in range(TB // 512)
                    ]
                    for k in range(KB):
                        for c in range(TB // 512):
                            nc.tensor.matmul(
                                gc[c][:],
                                wt[:, 0, f2, k * 128 : (k + 1) * 128],
                                xb[:, k, c * 512 : (c + 1) * 512],
                                start=(k == 0),
                                stop=(k == KB - 1),
                            )
                    for c in range(TB // 512):
                        sl = slice(c * 512, (c + 1) * 512)
                        nc.scalar.activation(
                            sil[:, sl], gc[c][:], mybir.ActivationFunctionType.Silu,
                            bias=bias0[:],
                        )

                    # u runs c-outer so each 512-col half of the product is
                    # ready as soon as its 8 k-accumulation matmuls retire
                    if f >= FB16:
                        pr, j = (f - FB16) // 2, (f - FB16) % 2
                        if j == 0:
                            ht8s[pr] = hpool.tile(
                                [128, 2, TB], fp8, tag=f"h8_{pr}",
                                name=f"h8_{t}_{pr}",
                            )
                        ht = None
                    else:
                        ht = hpool.tile([128, TB], f16, tag=f"h{f}")
                    for c in range(TB // 512):
                        sl = slice(c * 512, (c + 1) * 512)
                        u = psum.tile([128, 512], f32, tag=f"b{b + 2 + c}")
                        for k in range(KB):
                            nc.tensor.matmul(
                                u[:],
                                wt[:, 1, f2, k * 128 : (k + 1) * 128],
                                xb[:, k, sl],
                                start=(k == 0),
                                stop=(k == KB - 1),
                            )
                        # h tile = (sil * S_H) * u, written at the uniform
                        # stage-B operand scale in one DVE op
                        dst = ht[:, sl] if ht is not None else ht8s[pr][:, j, sl]
                        nc.vector.scalar_tensor_tensor(
                            dst, sil[:, sl], S_H, u[:], MULT, MULT,
                        )
                    if ht is not None:
                        hts.append(ht)

            if t + 1 < NT:
                # prefetch next block's x now: the trigger lands on the sync
                # ring AHEAD of this block's output-DMA triggers; its WAR on
                # the single-buffered tile resolves as stage B starts
                nxb = xpool.tile([128, KB, TB], f16, tag="xb")
                nc.sync.dma_start(nxb[:], xQ[t + 1])
                xbs[t + 1] = nxb

            # ---- stage B: out[tokens, h] += hT^T @ Wd, tokens on partitions
            # m-outer/f-inner; ALL chains (24 fp16 + 4 fp8 DoubleRow) land in
            # one bank at scale S_H*S_D, evicted by a single scaled Act copy
            for h2 in range(2):
                for m in range(8):
                    acc = psum.tile(
                        [128, 512], f32, tag=f"b{m}", name=f"acc{t}_{h2}_{m}"
                    )
                    msl = slice(m * 128, (m + 1) * 128)
                    for f in range(FB16):
                        nc.tensor.matmul(
                            acc[:],
                            hts[f][:, msl],
                            wdp[h2][:, f * 512 : (f + 1) * 512],
                            start=(f == 0),
                            stop=False,
                        )
                    for pr in range(NP8):
                        nc.tensor.matmul(
                            acc[:],
                            ht8s[pr][:, :, msl],
                            wd8p[pr][h2][:],
                            start=False,
                            stop=(pr == NP8 - 1),
                            perf_mode=DR,
                        )
                    ob = opool.tile([128, 512], f16, tag="ob")
                    nc.scalar.activation(
                        ob[:], acc[:], mybir.ActivationFunctionType.Copy,
                        scale=EVICT,
                    )
                    row = t * TB + m * 128
                    dst = out[row : row + 128, h2 * 512 : (h2 + 1) * 512]
                    eng = nc.scalar if m < 4 else nc.sync
                    eng.dma_start(dst, ob[:])

    nc.compile()
    return nc


def _get_module():
    if "nc" not in _CACHE:
        _CACHE["nc"] = _build_module()
    return _CACHE["nc"]


def _prep_inputs(hidden_states, Wg, Wu, Wd):
    f16 = np.float16
    x = np.asarray(hidden_states, dtype=np.float32).reshape(T, H)
    # xQ[t, p, k, tt] = x[t*TB+tt, 128k+p]
    xQ = np.ascontiguousarray(
        x.reshape(NT, TB, KB, 128).transpose(0, 3, 2, 1)
    ).astype(f16)
    in_maps = []
    for e in range(N_CORES):
        # w[f, p, (k m)] = W[e, 128k+p, 128f+m], f-major tiles of FQ slices
        def _wslices(W):
            return (
                np.asarray(W, dtype=np.float32)
                .reshape(KB, 128, FB, 128)
                .transpose(2, 1, 0, 3)
                .reshape(FB // FQ, FQ, 128, KB * 128)
                .transpose(0, 2, 1, 3)  # [8, 128, FQ, 1024]
            )
        wQ = np.ascontiguousarray(
            np.stack([_wslices(Wg[e]), _wslices(Wu[e])], axis=2)
        ).astype(f16)  # [8, 128, 2, FQ, 1024]
        wd_s = np.asarray(Wd[e], dtype=np.float32) * S_D
        wd_r = wd_s.reshape(FB, 128, 2, 512)
        # wdQ[h2, p, f*512+h] = Wd[e, 128f+p, 512h2+h] * S_D  (f < FB16)
        wdQ = np.ascontiguousarray(
            wd_r[:FB16].transpose(2, 1, 0, 3).reshape(2, 128, FB16 * 512)
        ).astype(f16)
        # wd8Q[pr, h2, p, j, hh] = e4m3 of the last NF8 slices
        wd8Q = np.clip(
            np.ascontiguousarray(
                wd_r[FB16:].reshape(NP8, 2, 128, 2, 512).transpose(0, 3, 2, 1, 4)
            ),
            -240.0, 240.0,
        ).astype(ml_dtypes.float8_e4m3)
        in_maps.append({"xQ": xQ, "wQ": wQ, "wdQ": wdQ, "wd8Q": wd8Q})
    return in_maps


def _run(in_maps, trace=False, **kwargs):
    from concourse import bass_utils

    nc = _get_module()
    return bass_utils.run_bass_kernel_spmd(
        nc, in_maps, core_ids=list(range(N_CORES)), trace=trace, **kwargs
    )


def kernel(hidden_states, Wg, Wu, Wd):
    import time

    in_maps = _prep_inputs(hidden_states, Wg, Wu, Wd)
    last_exc = None
    for attempt in range(3):
        try:
            res = _run(in_maps)
            break
        except Exception as exc:  # transient device-unrecoverable wedges
            last_exc = exc
            time.sleep(5 * (attempt + 1))
    else:
        raise last_exc
    partials = np.stack(
        [np.asarray(r["out"], dtype=np.float32) for r in res.results], axis=0
    )
    total = partials.sum(axis=0, dtype=np.float32)
    return total.reshape(2, 2048, H).astype(np.float32)
